# revision 1
# baseline (speedup 1.0000x reference)
"""Trainium2 Bass kernel for multi-head attention with RoPE (nn_Attention).

Reference computation (B=1, N=2048, D=1024, 16 heads, hd=64):
    q = x @ wq.T; k = x @ wk.T; v = x @ wv.T      (reshaped to heads)
    q, k = rope(q), rope(k)
    out = softmax(q k^T / sqrt(hd)) v              (non-causal, full)
    return (out reshaped) @ wp.T

Sharding: tensor-parallel over heads — each of the 8 cores owns 2 heads for
QKV projection + SDPA, then an AllToAll redistributes the attention output
so each core computes the final projection for its 256 sequence rows with
the full wp. Matmuls run in float32r (full-rate, ~1.7e-4 rel err).

Self-contained: only imports numpy + the concourse stack available in the
execution environment. kernel(**inputs) takes the full unsharded inputs and
returns the full output.
"""
import numpy as np

DIM = 1024
NHEADS = 16
HD = 64
SEQ = 2048
NCORES = 8
ROPE_BASE = 10000.0
HPC = NHEADS // NCORES      # heads per core = 2
CH = HPC * HD               # channels per core = 128
QCH = 512                   # q-chunk (free dim of S/P tiles)
NQC = SEQ // QCH            # 4
NKT = SEQ // 128            # 16 k-tiles
DCH = DIM // 128            # 8 contraction chunks

_CACHE = {}
_PARTS_MODE = "ab"


def _rope_tables():
    inv = 1.0 / (ROPE_BASE ** (np.arange(0, HD, 2, dtype=np.float64) / HD))
    t = np.arange(SEQ, dtype=np.float64)
    freqs = np.outer(t, inv)                      # [SEQ, 32]
    emb = np.concatenate([freqs, freqs], 1)       # [SEQ, 64]
    cosT = np.cos(emb).T                          # [64, SEQ]
    sinT = np.sin(emb).T
    sig = (np.arange(HD) + 32) % HD
    sT = sinT[sig]                                # shifted sin
    cos2 = np.concatenate([cosT, cosT], 0)        # [128, SEQ] (2 heads)
    s2 = np.concatenate([sT, sT], 0)
    return cos2, s2


def _r2t():
    # rotate-half matrix R (per head), block-diagonal over the 2 heads; we
    # pass R2.T as the stationary matmul operand.
    R = np.zeros((HD, HD), np.float64)
    for j in range(32):
        R[j, j + 32] = -1.0
        R[j + 32, j] = 1.0
    R2 = np.zeros((CH, CH), np.float64)
    R2[0:HD, 0:HD] = R
    R2[HD:CH, HD:CH] = R
    return np.ascontiguousarray(R2.T).astype(np.float32)


def _build(nrep=1, n_cores=NCORES, with_c=True, parts="ab"):
    global _PARTS_MODE
    _PARTS_MODE = parts
    import concourse.mybir as mybir
    import concourse.tile as tile
    from concourse import bacc
    from concourse.masks import make_identity

    F32 = mybir.dt.float32
    F32R = mybir.dt.float32r
    EXP = mybir.ActivationFunctionType.Exp

    nc = bacc.Bacc("TRN2", target_bir_lowering=False, debug=False,
                   num_devices=n_cores)

    xt_ext = nc.dram_tensor("xt", [DIM, SEQ], F32, kind="ExternalInput")
    wq_ext = nc.dram_tensor("wq_t", [DIM, CH], F32, kind="ExternalInput")
    wk_ext = nc.dram_tensor("wk_t", [DIM, CH], F32, kind="ExternalInput")
    wv_ext = nc.dram_tensor("wv_t", [DIM, CH], F32, kind="ExternalInput")
    wp_ext = nc.dram_tensor("wp_t", [DIM, DIM], F32, kind="ExternalInput")
    ck_ext = nc.dram_tensor("cos_k", [CH, SEQ], F32, kind="ExternalInput")
    sk_ext = nc.dram_tensor("sin_k", [CH, SEQ], F32, kind="ExternalInput")
    r2t_ext = nc.dram_tensor("r2t", [CH, CH], F32, kind="ExternalInput")
    out_ext = nc.dram_tensor("out", [SEQ // NCORES, DIM], F32,
                             kind="ExternalOutput")
    a2a_in = nc.dram_tensor("a2a_in", [NCORES, CH, SEQ // NCORES], F32)
    a2a_out = nc.dram_tensor("a2a_out", [NCORES, CH, SEQ // NCORES], F32)

    with tile.TileContext(nc) as tc:

        def stage_ab(Qp, Kp, Vsb, onescol, parts="ab"):
            # One unified scope for projections + attention so the Tile
            # scheduler can overlap attention chunks with later Q chunks.
            # PSUM budget (8 banks): big (2-bank slots x2) + small (1-bank
            # x2) + oaug (1-bank x2).
            with (
                tc.tile_pool(name="stA", bufs=1) as A_sb,
                tc.tile_pool(name="stA2", bufs=2) as A_db,
                tc.tile_pool(name="psBig", bufs=2, space="PSUM") as psBig,
                tc.tile_pool(name="psSm", bufs=2, space="PSUM") as psSm,
                tc.tile_pool(name="psO", bufs=2, space="PSUM") as psO,
                tc.tile_pool(name="stB", bufs=3) as B_db,
                tc.tile_pool(name="stBs", bufs=3) as B_sm,
            ):
                if parts in ("b", "s"):
                    _attention(Qp, Kp, Vsb, onescol, A_db, B_db, B_sm,
                               psBig, psSm, psO, None, None, None, False)
                    return
                aux1 = A_sb.tile([128, HD], F32, tag="aux1")
                nc.vector.memset(aux1[:], 1.0)
                nc.vector.tensor_copy(onescol[:], aux1[:])
                nc.vector.tensor_copy(
                    Vsb[:, :, :, HD],
                    aux1[:, 0:NKT * HPC].rearrange("p (k h) -> p k h", h=HPC))

                # ---- stage A inputs. float32r-typed DMAs are ~75x
                # slower than f32 on this platform, so everything lands in
                # f32 staging tiles and is round-copied to f32r on idle
                # engines (ACT for xt during the DMA lead-in, DVE for the
                # small weights).
                wq = A_sb.tile([128, DCH, CH], F32R, tag="wq")
                wk = A_sb.tile([128, DCH, CH], F32R, tag="wk")
                wv = A_sb.tile([128, DCH, CH], F32R, tag="wv")
                xt = A_sb.tile([128, DCH, SEQ], F32R, tag="xt")
                xt_r = xt_ext.rearrange("(c p) n -> p c n", p=128)
                r2t = A_sb.tile([CH, CH], F32R, tag="r2t")
                ck = A_sb.tile([CH, SEQ], F32, tag="ck")
                sk = A_sb.tile([CH, SEQ], F32, tag="sk")
                wkf = A_db.tile([128, DCH, CH], F32, tag="wf")
                nc.sync.dma_start(
                    out=wkf[:], in_=wk_ext.rearrange("(c p) j -> p c j", p=128))
                nc.vector.tensor_copy(wk[:], wkf[:])
                r2tf = A_db.tile([CH, CH], F32, tag="r2tf")
                nc.sync.dma_start(out=r2tf[:], in_=r2t_ext[:])
                nc.vector.tensor_copy(r2t[:], r2tf[:])
                wvf = A_db.tile([128, DCH, CH], F32, tag="wf")
                nc.sync.dma_start(
                    out=wvf[:], in_=wv_ext.rearrange("(c p) j -> p c j", p=128))
                nc.vector.tensor_copy(wv[:], wvf[:])
                for d in range(DCH):
                    xtf = A_db.tile([128, SEQ], F32, tag="xtf")
                    nc.sync.dma_start(out=xtf[:], in_=xt_r[:, d, :])
                    # round-robin the f32->f32r rounding copies over three
                    # engines so the copy chain is not serial on ACT
                    eng = (nc.scalar.copy, nc.vector.tensor_copy,
                           nc.gpsimd.tensor_copy)[d % 3]
                    eng(xt[:, d, :], xtf[:])
                nc.sync.dma_start(out=sk[:], in_=sk_ext[:])
                nc.sync.dma_start(out=ck[:], in_=ck_ext[:])
                wqf = A_db.tile([128, DCH, CH], F32, tag="wf")
                nc.sync.dma_start(
                    out=wqf[:], in_=wq_ext.rearrange("(c p) j -> p c j", p=128))
                nc.vector.tensor_copy(wq[:], wqf[:])
                ident = A_sb.tile([128, 128], F32, tag="ident")
                make_identity(nc, ident[:])
                identr = A_sb.tile([128, 128], F32R, tag="identr")
                nc.vector.tensor_copy(identr[:], ident[:])

                # ---- projections: K first, then V, then Q — attention
                # q-chunks only need Q' chunk-by-chunk, so emitting Q last
                # lets attention overlap the tail of the projections.
                def qk_proj(w_sb, cos_sb, sin_sb, dst, qc):
                    sl = slice(qc * QCH, (qc + 1) * QCH)
                    ps_q = psSm.tile([CH, QCH], F32, tag="sm")
                    for d in range(DCH):
                        nc.tensor.matmul(ps_q[:], w_sb[:, d, :],
                                         xt[:, d, sl],
                                         start=(d == 0), stop=(d == DCH - 1))
                    qs = A_db.tile([CH, QCH], F32R, tag="qs")
                    nc.vector.tensor_mul(qs[:], ps_q[:], sin_sb[:, sl])
                    qct = A_db.tile([CH, QCH], F32R, tag="qct")
                    nc.vector.tensor_mul(qct[:], ps_q[:], cos_sb[:, sl])
                    nc.tensor.matmul(ps_q[:], r2t[:], qs[:],
                                     start=True, stop=True)
                    nc.vector.tensor_add(dst[:, sl], qct[:], ps_q[:])

                # K and V projections, d-outer: all 8 chunk-accumulators
                # live at once (4 K halves in the two 2-bank "big" slots,
                # 4 V chunks in the four 1-bank slots), so the first xt
                # d-chunk to arrive immediately feeds 8 matmuls.
                kacc0 = psBig.tile([128, HPC, QCH], F32, tag="big")
                kacc1 = psBig.tile([128, HPC, QCH], F32, tag="big")
                vacc0 = psSm.tile([CH, QCH], F32, tag="sm")
                vacc1 = psSm.tile([CH, QCH], F32, tag="sm")
                vacc2 = psO.tile([CH, QCH], F32, tag="oaug")
                vacc3 = psO.tile([CH, QCH], F32, tag="oaug")
                kaccs = [kacc0[:, 0, :], kacc0[:, 1, :],
                         kacc1[:, 0, :], kacc1[:, 1, :]]
                vaccs = [vacc0, vacc1, vacc2, vacc3]
                for d in range(DCH):
                    st, sp = d == 0, d == DCH - 1
                    for c in range(NQC):
                        slc = slice(c * QCH, (c + 1) * QCH)
                        nc.tensor.matmul(kaccs[c], wk[:, d, :], xt[:, d, slc],
                                         start=st, stop=sp)
                        nc.tensor.matmul(vaccs[c][:], wv[:, d, :],
                                         xt[:, d, slc], start=st, stop=sp)

                # RoPE for K: the rot matmuls overwrite the K-accumulator
                # banks in place (start=True) after both DVE reads. The two
                # chunks of each accumulator tile are contiguous, so the DVE
                # muls/adds run at 1024 width (half the ops).
                for pair, kacc in ((0, kacc0), (1, kacc1)):
                    sl2 = slice(pair * 2 * QCH, (pair + 1) * 2 * QCH)
                    kview = kacc[:].rearrange("p a b -> p (a b)")
                    qs = A_db.tile([CH, 2 * QCH], F32R, tag="qs")
                    nc.vector.tensor_mul(qs[:], kview, sk[:, sl2])
                    qct = A_db.tile([CH, 2 * QCH], F32R, tag="qct")
                    nc.vector.tensor_mul(qct[:], kview, ck[:, sl2])
                    for half in range(2):
                        nc.tensor.matmul(
                            kacc[:, half, :], r2t[:],
                            qs[:, half * QCH:(half + 1) * QCH],
                            start=True, stop=True)
                    nc.vector.tensor_add(Kp[:, sl2], qct[:], kview)

                qk_proj(wq, ck, sk, Qp, 0)

                # V: copy out of psum, then PE-transpose into Vsb (emitted
                # after Q0 so the attention-critical path starts sooner)
                for c in range(NQC):
                    vt = A_db.tile([CH, QCH], F32R, tag="vt")
                    nc.scalar.copy(vt[:], vaccs[c][:])
                    for b in range(QCH // 128):
                        kti = c * (QCH // 128) + b
                        ps_t = psSm.tile([128, 128], F32R, tag="sm")
                        nc.tensor.transpose(
                            ps_t[:], vt[:, b * 128:(b + 1) * 128], identr[:])
                        nc.vector.tensor_copy(
                            Vsb[:, kti, :, 0:HD],
                            ps_t[:].rearrange("p (h j) -> p h j", h=HPC))

                if parts in ("ab", "b", "s"):
                    _attention(Qp, Kp, Vsb, onescol, A_db, B_db, B_sm,
                               psBig, psSm, psO, qk_proj, wq, (ck, sk), True)
                else:
                    for qc in range(1, NQC):
                        qk_proj(wq, ck, sk, Qp, qc)

        def _attention(Qp, Kp, Vsb, onescol, A_db, B_db, B_sm,
                       psBig, psSm, psO, qk_proj, wq, cs, interleave):
                # ---- attention per head pair, interleaved with the
                # projection of the next Q chunk (hides Q under exp) ----
                s_only = (_PARTS_MODE == "s")

                def emit_tail(o_ps, qc):
                    # softmax normalization + a2a scatter for chunk qc;
                    # deferred into the next chunk's exp shadow so the PE
                    # never stalls on the DVE reciprocal at a boundary.
                    for h in range(HPC):
                        rec = B_sm.tile([HD + 1, QCH], F32R, tag="rec")
                        with nc.allow_low_precision(
                                reason="f32r is fp32-width; rounding only"):
                            nc.vector.reciprocal(rec[HD:HD + 1, :],
                                                 o_ps[h][HD:HD + 1, :])
                        rb_ps = psSm.tile([HD, QCH], F32, tag="sm")
                        nc.tensor.matmul(rb_ps[:], onescol[HD:HD + 1, :],
                                         rec[HD:HD + 1, :],
                                         start=True, stop=True,
                                         tile_position=(HD, 0))
                        rb = B_sm.tile([HD, QCH], F32R, tag="rb_sb")
                        nc.vector.tensor_copy(rb[:], rb_ps[:])
                        on = B_db.tile([HD, QCH], F32, tag="on")
                        nc.vector.tensor_mul(on[:], o_ps[h][0:HD, :], rb[:])
                        # one strided DMA covers both destination cores
                        nc.sync.dma_start(
                            out=a2a_in[2 * qc:2 * qc + 2,
                                       h * HD:(h + 1) * HD, :]
                            .rearrange("r p n -> p r n"),
                            in_=on[:].rearrange("p (r n) -> p r n", r=2))

                pending = None
                for qc in range(NQC):
                    sl = slice(qc * QCH, (qc + 1) * QCH)
                    # software-pipelined emission: S(kt+1) is emitted
                    # before O(kt) so the in-order PE fills the exp(kt)
                    # shadow with the next S pair instead of stalling.
                    def emit_s(kt):
                        s_ps = psBig.tile([128, HPC, QCH], F32, tag="big")
                        for h in range(HPC):
                            nc.tensor.matmul(
                                s_ps[:, h, :],
                                Kp[h * HD:(h + 1) * HD,
                                   kt * 128:(kt + 1) * 128],
                                Qp[h * HD:(h + 1) * HD, sl],
                                start=True, stop=True,
                                tile_position=(h * HD, 0))
                        p_sb = B_db.tile([128, HPC, QCH], F32R, tag="p")
                        nc.scalar.activation(out=p_sb[:], in_=s_ps[:], func=EXP)
                        return p_sb

                    def emit_o(kt, p_sb):
                        for h in range(HPC):
                            nc.tensor.matmul(
                                o_ps[h][:], Vsb[:, kt, h, :], p_sb[:, h, :],
                                start=(kt == 0), stop=(kt == NKT - 1))

                    p_prev = emit_s(0)
                    if pending is not None:
                        emit_tail(*pending)
                        pending = None
                    if interleave and qc + 1 < NQC:
                        # Q(qc+1) projection rides in the exp shadows of this
                        # chunk's early k-tiles (emitted after S(0) so it
                        # cannot delay the attention-critical path).
                        qk_proj(wq, cs[0], cs[1], Qp, qc + 1)
                    o_ps = None
                    if not s_only:
                        o_ps0 = psO.tile([HD + 1, QCH], F32, tag="oaug")
                        o_ps1 = psO.tile([HD + 1, QCH], F32, tag="oaug")
                        o_ps = [o_ps0, o_ps1]
                    for kt in range(1, NKT):
                        p_cur = emit_s(kt)
                        if not s_only:
                            emit_o(kt - 1, p_prev)
                        p_prev = p_cur
                    if not s_only:
                        emit_o(NKT - 1, p_prev)
                        pending = (o_ps, qc)
                if pending is not None:
                    emit_tail(*pending)

        def stage_c():
            with (
                tc.tile_pool(name="stC", bufs=1) as C_sb,
                tc.tile_pool(name="stC2", bufs=2) as C_db,
                tc.tile_pool(name="psC", bufs=2, space="PSUM") as psC,
            ):
                wp = C_sb.tile([128, DCH, DIM], F32R, tag="wp")
                wpf = C_sb.tile([128, DCH, DIM], F32, tag="wpf")
                nc.sync.dma_start(
                    out=wpf[:], in_=wp_ext.rearrange("(s p) o -> p s o", p=128))
                nc.vector.tensor_copy(wp[:], wpf[:])
                nc.gpsimd.collective_compute(
                    "AllToAll", mybir.AluOpType.bypass,
                    replica_groups=[list(range(NCORES))],
                    ins=[a2a_in[:]], outs=[a2a_out[:]])
                gaf = C_sb.tile([CH, NCORES, 256], F32, tag="gaf")
                ga = C_sb.tile([CH, NCORES, 256], F32R, tag="ga")
                # per-src gather + rounding copy: the first projection
                # matmul starts after one 128KB chunk instead of the
                # whole 1MB (subtile deps gate per region)
                for r in range(NCORES):
                    nc.sync.dma_start(out=gaf[:, r, :], in_=a2a_out[r])
                    nc.vector.tensor_copy(ga[:, r, :], gaf[:, r, :])
                for nt in range(2):
                    for oc in range(2):
                        pp = psC.tile([128, 512], F32, tag="pp")
                        for src in range(NCORES):
                            nc.tensor.matmul(
                                pp[:], ga[:, src, nt * 128:(nt + 1) * 128],
                                wp[:, src, oc * 512:(oc + 1) * 512],
                                start=(src == 0), stop=(src == NCORES - 1))
                        ob = C_db.tile([128, 512], F32, tag="ob")
                        nc.scalar.copy(ob[:], pp[:])
                        nc.sync.dma_start(
                            out=out_ext[nt * 128:(nt + 1) * 128,
                                        oc * 512:(oc + 1) * 512],
                            in_=ob[:])

        with tc.tile_pool(name="persist", bufs=1) as P1:
            Qp = P1.tile([CH, SEQ], F32R, tag="Qp")
            Kp = P1.tile([CH, SEQ], F32R, tag="Kp")
            Vsb = P1.tile([128, NKT, HPC, HD + 1], F32R, tag="Vsb")
            onescol = P1.tile([128, HD], F32R, tag="onescol")
            if nrep == 1:
                if parts in ("b", "s"):
                    stage_ab(Qp, Kp, Vsb, onescol, "a")
                stage_ab(Qp, Kp, Vsb, onescol, parts)
                if with_c:
                    stage_c()
            else:
                # timing build: loop stages A+B (a collective inside a For_i
                # desyncs the mesh), run stage C once after the loop.
                if parts in ("b", "s"):
                    stage_ab(Qp, Kp, Vsb, onescol, "a")
                with tc.For_i(0, nrep, 1) as _i:
                    stage_ab(Qp, Kp, Vsb, onescol, parts)
                if with_c:
                    stage_c()

    nc.compile()
    return nc


def _get_nc(nrep=1, n_cores=NCORES, with_c=True, parts="ab"):
    key = ("nc", nrep, n_cores, with_c, parts)
    if key not in _CACHE:
        _CACHE[key] = _build(nrep, n_cores, with_c, parts)
    return _CACHE[key]


def _prep_in_maps(x, wq, wk, wv, wp):
    x2 = np.ascontiguousarray(np.asarray(x, np.float32).reshape(SEQ, DIM))
    xt = np.ascontiguousarray(x2.T)
    wq = np.asarray(wq, np.float32)
    wk = np.asarray(wk, np.float32)
    wv = np.asarray(wv, np.float32)
    wp = np.asarray(wp, np.float32)
    cos2, s2 = _rope_tables()
    scale = 1.0 / np.sqrt(HD)
    wq = wq * scale
    ck = np.ascontiguousarray(cos2).astype(np.float32)
    sk = np.ascontiguousarray(s2).astype(np.float32)
    r2t = _r2t()
    wpt = np.ascontiguousarray(wp.T)
    maps = []
    for c in range(NCORES):
        ch = slice(c * CH, (c + 1) * CH)
        maps.append({
            "xt": xt,
            "wq_t": np.ascontiguousarray(wq[ch, :].T),
            "wk_t": np.ascontiguousarray(wk[ch, :].T),
            "wv_t": np.ascontiguousarray(wv[ch, :].T),
            "wp_t": wpt,
            "cos_k": ck, "sin_k": sk,
            "r2t": r2t,
        })
    return maps


def kernel(x, wq, wk, wv, wp):
    from concourse.bass_utils import run_bass_kernel_spmd

    nc = _get_nc(1)
    maps = _prep_in_maps(x, wq, wk, wv, wp)
    res = run_bass_kernel_spmd(nc, maps, list(range(NCORES))).results
    out = np.concatenate([res[c]["out"] for c in range(NCORES)], axis=0)
    return out.reshape(1, SEQ, DIM).astype(np.float32)



# revision 13
# speedup vs baseline: 1.5411x; 1.5411x over previous
"""Trainium2 Bass kernel for multi-head attention with RoPE (nn_Attention).

Reference computation (B=1, N=2048, D=1024, 16 heads, hd=64):
    q = x @ wq.T; k = x @ wk.T; v = x @ wv.T      (reshaped to heads)
    q, k = rope(q), rope(k)
    out = softmax(q k^T / sqrt(hd)) v              (non-causal, full)
    return (out reshaped) @ wp.T

Sharding: tensor-parallel over heads — each of the 8 cores owns 2 heads for
QKV projection + SDPA, then an AllToAll redistributes the attention output
so each core computes the final projection for its 256 sequence rows with
the full wp.

v2: all matmul operands bf16 (host-precast), x streamed in 4 column blocks
so attention starts after the first block, ScalarE reserved exclusively for
exp, V transposed via the DMA xbar, wp prefetched during attention.

Self-contained: only imports numpy + the concourse stack available in the
execution environment. kernel(**inputs) takes the full unsharded inputs and
returns the full output.
"""
import numpy as np

DIM = 1024
NHEADS = 16
HD = 64
SEQ = 2048
NCORES = 8
ROPE_BASE = 10000.0
HPC = NHEADS // NCORES      # heads per core = 2
CH = HPC * HD               # channels per core = 128
QCH = 512                   # q-chunk (free dim of S/P tiles)
NQC = SEQ // QCH            # 4
NKT = SEQ // 128            # 16 k-tiles
DCH = DIM // 128            # 8 contraction chunks
NBLK = 4                    # x column blocks (512 seq cols each)

_CACHE = {}


def _rope_tables():
    inv = 1.0 / (ROPE_BASE ** (np.arange(0, HD, 2, dtype=np.float64) / HD))
    t = np.arange(SEQ, dtype=np.float64)
    freqs = np.outer(t, inv)                      # [SEQ, 32]
    emb = np.concatenate([freqs, freqs], 1)       # [SEQ, 64]
    cosT = np.cos(emb).T                          # [64, SEQ]
    sinT = np.sin(emb).T
    sig = (np.arange(HD) + 32) % HD
    sT = sinT[sig]                                # shifted sin
    cos2 = np.concatenate([cosT, cosT], 0)        # [128, SEQ] (2 heads)
    s2 = np.concatenate([sT, sT], 0)
    return cos2, s2


def _r2t():
    # rotate-half matrix R (per head), block-diagonal over the 2 heads; we
    # pass R2.T as the stationary matmul operand.
    R = np.zeros((HD, HD), np.float64)
    for j in range(32):
        R[j, j + 32] = -1.0
        R[j + 32, j] = 1.0
    R2 = np.zeros((CH, CH), np.float64)
    R2[0:HD, 0:HD] = R
    R2[HD:CH, HD:CH] = R
    return np.ascontiguousarray(R2.T)


def _build(nrep=1, n_cores=NCORES, with_c=True, parts="ab"):
    import concourse.mybir as mybir
    import concourse.tile as tile
    from concourse import bacc

    F32 = mybir.dt.float32
    F32R = mybir.dt.float32r
    BF16 = mybir.dt.bfloat16
    EXP = mybir.ActivationFunctionType.Exp

    nc = bacc.Bacc("TRN2", target_bir_lowering=False, debug=False,
                   num_devices=n_cores)

    xt_ext = nc.dram_tensor("xt", [DIM, SEQ], BF16, kind="ExternalInput")
    wq_ext = nc.dram_tensor("wq_t", [128, DCH * CH], BF16,
                            kind="ExternalInput")
    wk_ext = nc.dram_tensor("wk_t", [128, DCH * CH], BF16,
                            kind="ExternalInput")
    wv_ext = nc.dram_tensor("wv_t", [128, DCH * CH], BF16,
                            kind="ExternalInput")
    wp_ext = nc.dram_tensor("wp_t", [128, DCH * DIM], BF16,
                            kind="ExternalInput")
    ck_ext = nc.dram_tensor("cos_k", [CH, SEQ], BF16, kind="ExternalInput")
    sk_ext = nc.dram_tensor("sin_k", [CH, SEQ], BF16, kind="ExternalInput")
    r2t_ext = nc.dram_tensor("r2t", [CH, CH], BF16, kind="ExternalInput")
    out_ext = nc.dram_tensor("out", [SEQ // NCORES, DIM], F32,
                             kind="ExternalOutput")
    # Collective payload is bf16 on the wire, but the collective machinery
    # mishandles sub-4-byte dtypes, so the DRAM tensors are declared f32
    # (half the elements) and DMAs bitcast at the boundary.
    a2a_in = nc.dram_tensor("a2a_in", [NCORES, CH, SEQ // (2 * NCORES)], F32)
    a2a_out = nc.dram_tensor("a2a_out", [NCORES, CH, SEQ // (2 * NCORES)], F32)

    with tile.TileContext(nc) as tc:

        def stage_ab(Qp, Kp, Vsb, onescol, xt, wp):
            with (
                tc.tile_pool(name="stA", bufs=1) as A_sb,
                tc.tile_pool(name="stA2", bufs=2) as A_db,
                tc.tile_pool(name="psBig", bufs=2, space="PSUM") as psBig,
                tc.tile_pool(name="psSm", bufs=2, space="PSUM") as psSm,
                tc.tile_pool(name="psO", bufs=2, space="PSUM") as psO,
                tc.tile_pool(name="stB", bufs=4) as B_db,
                tc.tile_pool(name="stBs", bufs=3) as B_sm,
            ):
                aux1 = A_sb.tile([128, HD], F32, tag="aux1")
                nc.vector.memset(aux1[:], 1.0)
                # warm the ACT exp table at t=0 so the first real exp
                # doesn't eat the ~2.7us table load.
                warm = A_sb.tile([1, 8], F32, tag="warm")
                nc.scalar.activation(out=warm[:], in_=aux1[0:1, 0:8], func=EXP)
                nc.vector.tensor_copy(onescol[:], aux1[:])
                nc.vector.tensor_copy(
                    Vsb[:, :, :, HD],
                    aux1[:, 0:NKT * HPC].rearrange("p (k h) -> p k h", h=HPC))

                # ---- input DMAs (all bf16, single stream on sync queue),
                # ordered so the block-0 critical path (wk, xt block 0,
                # rope tables, wq) lands first.
                wk = A_sb.tile([128, DCH, CH], BF16, tag="wk")
                wq = A_sb.tile([128, DCH, CH], BF16, tag="wq")
                wv = A_sb.tile([128, DCH, CH], BF16, tag="wv")
                r2t = A_sb.tile([CH, CH], BF16, tag="r2t")
                ck = A_sb.tile([CH, SEQ], BF16, tag="ck")
                sk = A_sb.tile([CH, SEQ], BF16, tag="sk")
                xt_r = xt_ext.rearrange("(c p) n -> p c n", p=128)
                b0 = slice(0, QCH)
                nc.sync.dma_start(
                    out=wq[:], in_=wq_ext.rearrange("p (c j) -> p c j", j=CH))
                nc.sync.dma_start(
                    out=wk[:], in_=wk_ext.rearrange("p (c j) -> p c j", j=CH))
                for d in range(DCH):
                    nc.sync.dma_start(out=xt[:, d, b0], in_=xt_r[:, d, b0])
                nc.sync.dma_start(out=r2t[:], in_=r2t_ext[:])
                nc.sync.dma_start(out=sk[:, b0], in_=sk_ext[:, b0])
                nc.sync.dma_start(out=ck[:, b0], in_=ck_ext[:, b0])
                nc.sync.dma_start(
                    out=wv[:], in_=wv_ext.rearrange("p (c j) -> p c j", j=CH))
                rest = slice(QCH, SEQ)
                nc.sync.dma_start(out=sk[:, rest], in_=sk_ext[:, rest])
                nc.sync.dma_start(out=ck[:, rest], in_=ck_ext[:, rest])
                for b in range(1, NBLK):
                    sl = slice(b * QCH, (b + 1) * QCH)
                    nc.sync.dma_start(out=xt[:, :, sl], in_=xt_r[:, :, sl])
                if with_c:
                    # prefetch wp for stage C behind the x blocks
                    nc.sync.dma_start(
                        out=wp[:],
                        in_=wp_ext.rearrange("p (s o) -> p s o", o=DIM))

                def rope_to(acc_ps, dst, sl, width):
                    # dst[:, sl] = acc*cos + R2T @ (acc*sin), overwriting
                    # acc_ps in place for the rotation matmul.
                    qs = A_db.tile([CH, width], BF16, tag="qs")
                    nc.vector.tensor_mul(qs[:], acc_ps[:], sk[:, sl])
                    qct = A_db.tile([CH, width], BF16, tag="qct")
                    nc.vector.tensor_mul(qct[:], acc_ps[:], ck[:, sl])
                    nc.tensor.matmul(acc_ps[:], r2t[:], qs[:],
                                     start=True, stop=True)
                    nc.vector.tensor_add(dst[:, sl], qct[:], acc_ps[:])

                def proj_units(w_sb, dst, b):
                    # split a 1024-contraction projection + rope into 3
                    # emission units so it can be paced between S steps.
                    sl = slice(b * QCH, (b + 1) * QCH)
                    box = {}

                    def u0():
                        box["ps"] = psSm.tile([CH, QCH], F32, tag="sm", name="ps")
                        for d in range(4):
                            nc.tensor.matmul(box["ps"][:], w_sb[:, d, :],
                                             xt[:, d, sl],
                                             start=(d == 0), stop=False)

                    def u1():
                        for d in range(4, DCH):
                            nc.tensor.matmul(box["ps"][:], w_sb[:, d, :],
                                             xt[:, d, sl],
                                             start=False, stop=(d == DCH - 1))

                    def u2():
                        rope_to(box["ps"], dst, sl, QCH)

                    return [u0, u1, u2]

                def v_units(b):
                    sl = slice(b * QCH, (b + 1) * QCH)
                    box = {}

                    def u0():
                        box["ps"] = psSm.tile([CH, QCH], F32, tag="sm", name="ps")
                        for d in range(4):
                            nc.tensor.matmul(box["ps"][:], wv[:, d, :],
                                             xt[:, d, sl],
                                             start=(d == 0), stop=False)

                    def u1():
                        for d in range(4, DCH):
                            nc.tensor.matmul(box["ps"][:], wv[:, d, :],
                                             xt[:, d, sl],
                                             start=False, stop=(d == DCH - 1))
                        vt = B_db.tile([CH, QCH], BF16, tag="vt",
                                       name="vt")
                        nc.vector.tensor_copy(vt[:], box["ps"][:])
                        box["vt"] = vt

                    def tr(i):
                        def u():
                            kti = b * (QCH // 128) + i
                            tmp = B_sm.tile([128, 128], BF16, tag="vtr")
                            nc.sync.dma_start(
                                out=tmp[:],
                                in_=box["vt"][:, i * 128:(i + 1) * 128],
                                transpose=True)
                            nc.vector.tensor_copy(
                                Vsb[:, kti, :, 0:HD],
                                tmp[:].rearrange("p (h j) -> p h j", h=HPC))
                        return u

                    return [u0, u1, tr(0), tr(1), tr(2), tr(3)]

                def run_units(units):
                    for u in units:
                        u()

                def emit_tail(o_ps, qc):
                    # softmax normalization + a2a scatter for chunk qc;
                    # deferred into the next chunk's exp shadow.
                    for h in range(HPC):
                        rec = B_sm.tile([HD + 1, QCH], F32R, tag="rec")
                        with nc.allow_low_precision(
                                reason="f32r is fp32-width; rounding only"):
                            nc.vector.reciprocal(rec[HD:HD + 1, :],
                                                 o_ps[h][HD:HD + 1, :])
                        rb_ps = psSm.tile([HD, QCH], F32, tag="sm")
                        nc.tensor.matmul(rb_ps[:], onescol[HD:HD + 1, :],
                                         rec[HD:HD + 1, :],
                                         start=True, stop=True,
                                         tile_position=(HD, 0))
                        rb = B_sm.tile([HD, QCH], BF16, tag="rb_sb")
                        nc.vector.tensor_copy(rb[:], rb_ps[:])
                        on = B_db.tile([HD, QCH], BF16, tag="on")
                        nc.vector.tensor_mul(on[:], o_ps[h][0:HD, :], rb[:])
                        # one strided DMA covers both destination cores
                        nc.sync.dma_start(
                            out=a2a_in.bitcast(BF16)[2 * qc:2 * qc + 2,
                                                     h * HD:(h + 1) * HD, :]
                            .rearrange("r p n -> p r n"),
                            in_=on[:].rearrange("p (r n) -> p r n", r=2))

                def emit_s(qc, kt):
                    sl = slice(qc * QCH, (qc + 1) * QCH)
                    s_ps = psBig.tile([128, HPC, QCH], F32, tag="big")
                    for h in range(HPC):
                        nc.tensor.matmul(
                            s_ps[:, h, :],
                            Kp[h * HD:(h + 1) * HD,
                               kt * 128:(kt + 1) * 128],
                            Qp[h * HD:(h + 1) * HD, sl],
                            start=True, stop=True,
                            tile_position=(h * HD, 0))
                    p_sb = B_db.tile([128, HPC, QCH], BF16, tag="p")
                    nc.scalar.activation(out=p_sb[:], in_=s_ps[:], func=EXP)
                    return p_sb

                o_tiles = {}

                def emit_o(qc, kt, p_sb):
                    if kt == 0:
                        # allocated here (not at S-emission) so the psO
                        # slot-reuse dependency sees the previous chunk's
                        # tail reads, which are emitted before this point.
                        o0 = psO.tile([HD + 1, QCH], F32, tag="oaug")
                        o1 = psO.tile([HD + 1, QCH], F32, tag="oaug")
                        o_tiles[qc] = [o0, o1]
                    for h in range(HPC):
                        nc.tensor.matmul(
                            o_tiles[qc][h][:], Vsb[:, kt, h, :],
                            p_sb[:, h, :],
                            start=(kt == 0), stop=(kt == NKT - 1))

                # ---- lead-in: block-0 K, Q(0), V(0) straight away.
                # (V transposes cost no PE time; emit them here too.)
                run_units(proj_units(wq, Qp, 0))
                run_units(proj_units(wk, Kp, 0))
                run_units(v_units(0))

                # background emission units, drained between S steps.
                # Order respects data deadlines: O(0, kt) is emitted at
                # step kt+2, S(0, kt) at step kt, Q(qc) before step 16*qc.
                bg = []
                bg += proj_units(wk, Kp, 1)      # K(1): before step 4
                bg += v_units(1)                 # V(1): before step 6
                bg += proj_units(wk, Kp, 2)      # K(2): before step 8
                bg += v_units(2)                 # V(2): before step 10
                bg += proj_units(wk, Kp, 3)      # K(3): before step 12
                bg += v_units(3)                 # V(3): before step 14
                bg += proj_units(wq, Qp, 1)      # Q(1): before step 16
                bg += proj_units(wq, Qp, 2)      # Q(2): before step 32
                bg += proj_units(wq, Qp, 3)      # Q(3): before step 48
                bg.reverse()                     # pop() from the end
                DRAIN = {0: 2, 1: 2, 2: 2, 3: 2}

                # Global software pipeline over the 64 (qc, kt) steps:
                # O-emission runs 2 steps behind S/exp; at a chunk's last
                # k-tile we catch up, pre-emit the next chunk's first S so
                # the exp stream never waits on the tail, then emit the
                # tail (whose reads land before the next chunk's first O
                # allocates the psO slots).
                p_fifo = []          # [(qc, kt, p_sb)] not yet O-consumed
                emitted = set()

                def s_step(step):
                    if step in emitted or step >= NQC * NKT:
                        return
                    emitted.add(step)
                    qc, kt = divmod(step, NKT)
                    p_fifo.append((qc, kt, emit_s(qc, kt)))

                for step in range(NQC * NKT):
                    qc, kt = divmod(step, NKT)
                    s_step(step)
                    if kt == NKT - 1:
                        while len(p_fifo) > 1:
                            emit_o(*p_fifo.pop(0))
                        s_step(step + 1)
                        emit_o(*p_fifo.pop(0))
                        emit_tail(o_tiles[qc], qc)
                    else:
                        while len(p_fifo) > 2:
                            emit_o(*p_fifo.pop(0))
                        for _ in range(DRAIN[qc]):
                            if bg:
                                bg.pop()()
                while bg:
                    bg.pop()()

        def stage_c(wp):
            with (
                tc.tile_pool(name="stC", bufs=1) as C_sb,
                tc.tile_pool(name="stC2", bufs=2) as C_db,
                tc.tile_pool(name="psC", bufs=2, space="PSUM") as psC,
            ):
                nc.gpsimd.collective_compute(
                    "AllToAll", mybir.AluOpType.bypass,
                    replica_groups=[list(range(NCORES))],
                    ins=[a2a_in[:]], outs=[a2a_out[:]])
                ga = C_sb.tile([CH, NCORES, 256], BF16, tag="ga")
                # per-src gather: the first projection matmul starts after
                # one chunk instead of the whole payload
                for r in range(NCORES):
                    nc.sync.dma_start(out=ga[:, r, :],
                                      in_=a2a_out.bitcast(BF16)[r])
                for nt in range(2):
                    for oc in range(2):
                        pp = psC.tile([128, 512], F32, tag="pp")
                        for src in range(NCORES):
                            nc.tensor.matmul(
                                pp[:], ga[:, src, nt * 128:(nt + 1) * 128],
                                wp[:, src, oc * 512:(oc + 1) * 512],
                                start=(src == 0), stop=(src == NCORES - 1))
                        ob = C_db.tile([128, 512], F32, tag="ob")
                        nc.vector.tensor_copy(ob[:], pp[:])
                        nc.sync.dma_start(
                            out=out_ext[nt * 128:(nt + 1) * 128,
                                        oc * 512:(oc + 1) * 512],
                            in_=ob[:])

        with tc.tile_pool(name="persist", bufs=1) as P1:
            Qp = P1.tile([CH, SEQ], BF16, tag="Qp")
            Kp = P1.tile([CH, SEQ], BF16, tag="Kp")
            Vsb = P1.tile([128, NKT, HPC, HD + 1], BF16, tag="Vsb")
            onescol = P1.tile([128, HD], F32R, tag="onescol")
            xt = P1.tile([128, DCH, SEQ], BF16, tag="xt")
            wp = (P1.tile([128, DCH, DIM], BF16, tag="wp", name="wp")
                  if with_c else None)
            if nrep == 1:
                stage_ab(Qp, Kp, Vsb, onescol, xt, wp)
                if with_c:
                    stage_c(wp)
            else:
                # timing build: loop stages A+B (a collective inside a For_i
                # desyncs the mesh), run stage C once after the loop.
                with tc.For_i(0, nrep, 1) as _i:
                    stage_ab(Qp, Kp, Vsb, onescol, xt, wp)
                if with_c:
                    stage_c(wp)

    nc.compile()
    return nc


def _get_nc(nrep=1, n_cores=NCORES, with_c=True, parts="ab"):
    key = ("nc", nrep, n_cores, with_c, parts)
    if key not in _CACHE:
        _CACHE[key] = _build(nrep, n_cores, with_c, parts)
    return _CACHE[key]


def _prep_in_maps(x, wq, wk, wv, wp):
    import ml_dtypes

    bf16 = ml_dtypes.bfloat16
    x2 = np.ascontiguousarray(np.asarray(x, np.float32).reshape(SEQ, DIM))
    xt = np.ascontiguousarray(x2.T).astype(bf16)
    wq = np.asarray(wq, np.float64)
    wk = np.asarray(wk, np.float64)
    wv = np.asarray(wv, np.float64)
    wp = np.asarray(wp, np.float32)
    cos2, s2 = _rope_tables()
    scale = 1.0 / np.sqrt(HD)
    wq = wq * scale
    ck = np.ascontiguousarray(cos2).astype(bf16)
    sk = np.ascontiguousarray(s2).astype(bf16)
    r2t = _r2t().astype(bf16)
    wpt = np.ascontiguousarray(wp.T)
    def wlay(w_t):
        # [DIM, CH] -> [128 part, DCH*CH] so the DMA is contiguous
        return np.ascontiguousarray(
            w_t.reshape(DCH, 128, CH).transpose(1, 0, 2).reshape(
                128, DCH * CH)).astype(bf16)

    wp_l = np.ascontiguousarray(
        wpt.reshape(DCH, 128, DIM).transpose(1, 0, 2).reshape(
            128, DCH * DIM)).astype(bf16)
    maps = []
    for c in range(NCORES):
        ch = slice(c * CH, (c + 1) * CH)
        maps.append({
            "xt": xt,
            "wq_t": wlay(np.ascontiguousarray(wq[ch, :].T)),
            "wk_t": wlay(np.ascontiguousarray(wk[ch, :].T)),
            "wv_t": wlay(np.ascontiguousarray(wv[ch, :].T)),
            "wp_t": wp_l,
            "cos_k": ck, "sin_k": sk,
            "r2t": r2t,
        })
    return maps


def kernel(x, wq, wk, wv, wp):
    from concourse.bass_utils import run_bass_kernel_spmd

    nc = _get_nc(1)
    maps = _prep_in_maps(x, wq, wk, wv, wp)
    res = run_bass_kernel_spmd(nc, maps, list(range(NCORES))).results
    out = np.concatenate([res[c]["out"] for c in range(NCORES)], axis=0)
    return out.reshape(1, SEQ, DIM).astype(np.float32)


# revision 16
# speedup vs baseline: 2.2614x; 1.4674x over previous
"""Trainium2 Bass kernel for multi-head attention with RoPE (nn_Attention).

Reference computation (B=1, N=2048, D=1024, 16 heads, hd=64):
    q = x @ wq.T; k = x @ wk.T; v = x @ wv.T      (reshaped to heads)
    q, k = rope(q), rope(k)
    out = softmax(q k^T / sqrt(hd)) v              (non-causal, full)
    return (out reshaped) @ wp.T

Sharding: tensor-parallel over heads — each of the 8 cores owns 2 heads for
QKV projection + SDPA, then an AllToAll redistributes the attention output
so each core computes the final projection for its 256 sequence rows with
the full wp.

v2: all matmul operands bf16 (host-precast), x streamed in 4 column blocks
so attention starts after the first block, ScalarE reserved exclusively for
exp, V transposed via the DMA xbar, wp prefetched during attention.

Self-contained: only imports numpy + the concourse stack available in the
execution environment. kernel(**inputs) takes the full unsharded inputs and
returns the full output.
"""
import numpy as np

DIM = 1024
NHEADS = 16
HD = 64
SEQ = 2048
NCORES = 8
ROPE_BASE = 10000.0
HPC = NHEADS // NCORES      # heads per core = 2
CH = HPC * HD               # channels per core = 128
QCH = 512                   # q-chunk (free dim of S/P tiles)
NQC = SEQ // QCH            # 4
NKT = SEQ // 128            # 16 k-tiles
DCH = DIM // 128            # 8 contraction chunks
NBLK = 4                    # x column blocks (512 seq cols each)

_CACHE = {}


def _rope_tables():
    inv = 1.0 / (ROPE_BASE ** (np.arange(0, HD, 2, dtype=np.float64) / HD))
    t = np.arange(SEQ, dtype=np.float64)
    freqs = np.outer(t, inv)                      # [SEQ, 32]
    emb = np.concatenate([freqs, freqs], 1)       # [SEQ, 64]
    cosT = np.cos(emb).T                          # [64, SEQ]
    sinT = np.sin(emb).T
    sig = (np.arange(HD) + 32) % HD
    sT = sinT[sig]                                # shifted sin
    cos2 = np.concatenate([cosT, cosT], 0)        # [128, SEQ] (2 heads)
    s2 = np.concatenate([sT, sT], 0)
    return cos2, s2


def _r2t():
    # rotate-half matrix R (per head), block-diagonal over the 2 heads; we
    # pass R2.T as the stationary matmul operand.
    R = np.zeros((HD, HD), np.float64)
    for j in range(32):
        R[j, j + 32] = -1.0
        R[j + 32, j] = 1.0
    R2 = np.zeros((CH, CH), np.float64)
    R2[0:HD, 0:HD] = R
    R2[HD:CH, HD:CH] = R
    return np.ascontiguousarray(R2.T)


def _build(nrep=1, n_cores=NCORES, with_c=True, parts="ab"):
    import concourse.mybir as mybir
    import concourse.tile as tile
    from concourse import bacc

    F32 = mybir.dt.float32
    F32R = mybir.dt.float32r
    BF16 = mybir.dt.bfloat16
    EXP = mybir.ActivationFunctionType.Exp

    nc = bacc.Bacc("TRN2", target_bir_lowering=False, debug=False,
                   num_devices=n_cores)

    xt_ext = nc.dram_tensor("xt", [DIM, SEQ], BF16, kind="ExternalInput")
    wq_ext = nc.dram_tensor("wq_t", [128, DCH * CH], BF16,
                            kind="ExternalInput")
    wk_ext = nc.dram_tensor("wk_t", [128, DCH * CH], BF16,
                            kind="ExternalInput")
    wv_ext = nc.dram_tensor("wv_t", [128, DCH * CH], BF16,
                            kind="ExternalInput")
    wp_ext = nc.dram_tensor("wp_t", [128, DCH * DIM], BF16,
                            kind="ExternalInput")
    ck_ext = nc.dram_tensor("cos_k", [CH, SEQ], BF16, kind="ExternalInput")
    sk_ext = nc.dram_tensor("sin_k", [CH, SEQ], BF16, kind="ExternalInput")
    r2t_ext = nc.dram_tensor("r2t", [CH, CH], BF16, kind="ExternalInput")
    sel_ext = nc.dram_tensor("sel", [NCORES, NHEADS, 128], BF16,
                             kind="ExternalInput")
    out_ext = nc.dram_tensor("out", [SEQ // NCORES, DIM], F32,
                             kind="ExternalOutput")
    # Collective payload is bf16 on the wire, but the collective machinery
    # mishandles sub-4-byte dtypes, so the DRAM tensors are declared f32
    # (half the elements) and DMAs bitcast at the boundary. Each (head,
    # dst) slot carries the unnormalized O (64 rows) plus the softmax
    # denominator row (row 64); the receiver normalizes after the AllToAll.
    a2a_in = nc.dram_tensor(
        "a2a_in", [NCORES, HPC, HD + 1, SEQ // (2 * NCORES)], F32)
    a2a_out = nc.dram_tensor(
        "a2a_out", [NCORES, HPC, HD + 1, SEQ // (2 * NCORES)], F32)

    with tile.TileContext(nc) as tc:

        def stage_ab(Qp, Kp, Vsb, sel, xt, wp, parts="ab"):
            with (
                tc.tile_pool(name="stA", bufs=1) as A_sb,
                tc.tile_pool(name="stA2", bufs=2) as A_db,
                tc.tile_pool(name="psBig", bufs=2, space="PSUM") as psBig,
                tc.tile_pool(name="psSm", bufs=2, space="PSUM") as psSm,
                tc.tile_pool(name="psO", bufs=2, space="PSUM") as psO,
                tc.tile_pool(name="stB", bufs=4) as B_db,
                tc.tile_pool(name="stBs", bufs=3) as B_sm,
            ):
                aux1 = A_sb.tile([128, HD], F32, tag="aux1")
                nc.vector.memset(aux1[:], 1.0)
                # warm the ACT exp table at t=0 so the first real exp
                # doesn't eat the ~2.7us table load.
                warm = A_sb.tile([1, 8], F32, tag="warm")
                nc.scalar.activation(out=warm[:], in_=aux1[0:1, 0:8], func=EXP)
                nc.vector.tensor_copy(
                    Vsb[:, :, :, HD],
                    aux1[:, 0:NKT * HPC].rearrange("p (k h) -> p k h", h=HPC))

                # ---- input DMAs (all bf16, single stream on sync queue),
                # ordered so the block-0 critical path (wk, xt block 0,
                # rope tables, wq) lands first.
                wk = A_sb.tile([128, DCH, CH], BF16, tag="wk")
                wq = A_sb.tile([128, DCH, CH], BF16, tag="wq")
                wv = A_sb.tile([128, DCH, CH], BF16, tag="wv")
                r2t = A_sb.tile([CH, CH], BF16, tag="r2t")
                ck = A_sb.tile([CH, SEQ], BF16, tag="ck")
                sk = A_sb.tile([CH, SEQ], BF16, tag="sk")
                xt_r = xt_ext.rearrange("(c p) n -> p c n", p=128)
                b0 = slice(0, QCH)
                nc.sync.dma_start(
                    out=wq[:], in_=wq_ext.rearrange("p (c j) -> p c j", j=CH))
                nc.sync.dma_start(
                    out=wk[:], in_=wk_ext.rearrange("p (c j) -> p c j", j=CH))
                for d in range(DCH):
                    nc.sync.dma_start(out=xt[:, d, b0], in_=xt_r[:, d, b0])
                nc.sync.dma_start(out=r2t[:], in_=r2t_ext[:])
                if with_c:
                    nc.sync.dma_start(
                        out=sel[:], in_=sel_ext.rearrange("s r p -> r s p"))
                nc.sync.dma_start(out=sk[:, b0], in_=sk_ext[:, b0])
                nc.sync.dma_start(out=ck[:, b0], in_=ck_ext[:, b0])
                nc.sync.dma_start(
                    out=wv[:], in_=wv_ext.rearrange("p (c j) -> p c j", j=CH))
                rest = slice(QCH, SEQ)
                nc.sync.dma_start(out=sk[:, rest], in_=sk_ext[:, rest])
                nc.sync.dma_start(out=ck[:, rest], in_=ck_ext[:, rest])
                for b in range(1, NBLK):
                    sl = slice(b * QCH, (b + 1) * QCH)
                    nc.sync.dma_start(out=xt[:, :, sl], in_=xt_r[:, :, sl])
                if with_c:
                    # prefetch wp for stage C behind the x blocks
                    nc.sync.dma_start(
                        out=wp[:],
                        in_=wp_ext.rearrange("p (s o) -> p s o", o=DIM))

                def rope_to(acc_ps, dst, sl, width):
                    # dst[:, sl] = acc*cos + R2T @ (acc*sin), overwriting
                    # acc_ps in place for the rotation matmul.
                    qs = A_db.tile([CH, width], BF16, tag="qs")
                    nc.vector.tensor_mul(qs[:], acc_ps[:], sk[:, sl])
                    qct = A_db.tile([CH, width], BF16, tag="qct")
                    nc.vector.tensor_mul(qct[:], acc_ps[:], ck[:, sl])
                    nc.tensor.matmul(acc_ps[:], r2t[:], qs[:],
                                     start=True, stop=True)
                    nc.vector.tensor_add(dst[:, sl], qct[:], acc_ps[:])

                def proj_units(w_sb, dst, b):
                    # split a 1024-contraction projection + rope into 3
                    # emission units so it can be paced between S steps.
                    sl = slice(b * QCH, (b + 1) * QCH)
                    box = {}

                    def u0():
                        box["ps"] = psSm.tile([CH, QCH], F32, tag="sm", name="ps")
                        for d in range(4):
                            nc.tensor.matmul(box["ps"][:], w_sb[:, d, :],
                                             xt[:, d, sl],
                                             start=(d == 0), stop=False)

                    def u1():
                        for d in range(4, DCH):
                            nc.tensor.matmul(box["ps"][:], w_sb[:, d, :],
                                             xt[:, d, sl],
                                             start=False, stop=(d == DCH - 1))

                    def u2():
                        rope_to(box["ps"], dst, sl, QCH)

                    return [u0, u1, u2]

                def v_units(b):
                    sl = slice(b * QCH, (b + 1) * QCH)
                    box = {}

                    def u0():
                        box["ps"] = psSm.tile([CH, QCH], F32, tag="sm", name="ps")
                        for d in range(4):
                            nc.tensor.matmul(box["ps"][:], wv[:, d, :],
                                             xt[:, d, sl],
                                             start=(d == 0), stop=False)

                    def u1():
                        for d in range(4, DCH):
                            nc.tensor.matmul(box["ps"][:], wv[:, d, :],
                                             xt[:, d, sl],
                                             start=False, stop=(d == DCH - 1))
                        vt = B_db.tile([CH, QCH], BF16, tag="vt",
                                       name="vt")
                        nc.vector.tensor_copy(vt[:], box["ps"][:])
                        box["vt"] = vt

                    def tr(i):
                        def u():
                            kti = b * (QCH // 128) + i
                            tmp = B_sm.tile([128, 128], BF16, tag="vtr")
                            nc.sync.dma_start(
                                out=tmp[:],
                                in_=box["vt"][:, i * 128:(i + 1) * 128],
                                transpose=True)
                            nc.vector.tensor_copy(
                                Vsb[:, kti, :, 0:HD],
                                tmp[:].rearrange("p (h j) -> p h j", h=HPC))
                        return u

                    return [u0, u1, tr(0), tr(1), tr(2), tr(3)]

                def run_units(units):
                    for u in units:
                        u()

                def emit_tail(o_ps, qc):
                    # scatter the unnormalized O + denominator row for
                    # chunk qc (normalization happens on the receiver).
                    for h in range(HPC):
                        on = B_db.tile([HD + 1, QCH], BF16, tag="on")
                        nc.vector.tensor_copy(on[:], o_ps[h][:])
                        # one strided DMA covers both destination cores
                        nc.sync.dma_start(
                            out=a2a_in.bitcast(BF16)[2 * qc:2 * qc + 2,
                                                     h, :, :]
                            .rearrange("r p n -> p r n"),
                            in_=on[:].rearrange("p (r n) -> p r n", r=2))

                def emit_s(qc, kt):
                    sl = slice(qc * QCH, (qc + 1) * QCH)
                    s_ps = psBig.tile([128, HPC, QCH], F32, tag="big")
                    for h in range(HPC):
                        nc.tensor.matmul(
                            s_ps[:, h, :],
                            Kp[h * HD:(h + 1) * HD,
                               kt * 128:(kt + 1) * 128],
                            Qp[h * HD:(h + 1) * HD, sl],
                            start=True, stop=True,
                            tile_position=(h * HD, 0))
                    p_sb = B_db.tile([128, HPC, QCH], BF16, tag="p")
                    nc.scalar.activation(out=p_sb[:], in_=s_ps[:], func=EXP)
                    return p_sb

                o_tiles = {}

                def emit_o(qc, kt, p_sb):
                    if kt == 0:
                        # allocated here (not at S-emission) so the psO
                        # slot-reuse dependency sees the previous chunk's
                        # tail reads, which are emitted before this point.
                        o0 = psO.tile([HD + 1, QCH], F32, tag="oaug")
                        o1 = psO.tile([HD + 1, QCH], F32, tag="oaug")
                        o_tiles[qc] = [o0, o1]
                    for h in range(HPC):
                        nc.tensor.matmul(
                            o_tiles[qc][h][:], Vsb[:, kt, h, :],
                            p_sb[:, h, :],
                            start=(kt == 0), stop=(kt == NKT - 1))

                # ---- lead-in: block-0 K, Q(0), V(0) straight away.
                # (V transposes cost no PE time; emit them here too.)
                run_units(proj_units(wq, Qp, 0))
                run_units(proj_units(wk, Kp, 0))
                run_units(v_units(0))

                # background emission units, drained between S steps.
                # Order respects data deadlines: O(0, kt) is emitted at
                # step kt+2, S(0, kt) at step kt, Q(qc) before step 16*qc.
                bg = []
                bg += proj_units(wk, Kp, 1)      # K(1): before step 4
                bg += v_units(1)                 # V(1): before step 6
                bg += proj_units(wk, Kp, 2)      # K(2): before step 8
                bg += v_units(2)                 # V(2): before step 10
                bg += proj_units(wk, Kp, 3)      # K(3): before step 12
                bg += v_units(3)                 # V(3): before step 14
                bg += proj_units(wq, Qp, 1)      # Q(1): before step 16
                bg += proj_units(wq, Qp, 2)      # Q(2): before step 32
                bg += proj_units(wq, Qp, 3)      # Q(3): before step 48
                bg.reverse()                     # pop() from the end
                DRAIN = {0: 2, 1: 2, 2: 2, 3: 2}

                if parts == "a":
                    while bg:
                        bg.pop()()
                    return

                # Global software pipeline over the 64 (qc, kt) steps:
                # O-emission runs 2 steps behind S/exp; at a chunk's last
                # k-tile we catch up, pre-emit the next chunk's first S so
                # the exp stream never waits on the tail, then emit the
                # tail (whose reads land before the next chunk's first O
                # allocates the psO slots).
                p_fifo = []          # [(qc, kt, p_sb)] not yet O-consumed
                emitted = set()

                def s_step(step):
                    if step in emitted or step >= NQC * NKT:
                        return
                    emitted.add(step)
                    qc, kt = divmod(step, NKT)
                    p_fifo.append((qc, kt, emit_s(qc, kt)))

                s_only = parts == "s"
                for step in range(NQC * NKT):
                    qc, kt = divmod(step, NKT)
                    s_step(step)
                    if kt == NKT - 1:
                        while len(p_fifo) > 1:
                            if not s_only:
                                emit_o(*p_fifo.pop(0))
                            else:
                                p_fifo.pop(0)
                        s_step(step + 1)
                        if not s_only:
                            emit_o(*p_fifo.pop(0))
                            emit_tail(o_tiles[qc], qc)
                        else:
                            p_fifo.pop(0)
                    else:
                        while len(p_fifo) > 2:
                            if not s_only:
                                emit_o(*p_fifo.pop(0))
                            else:
                                p_fifo.pop(0)
                        for _ in range(DRAIN[qc]):
                            if bg:
                                bg.pop()()
                while bg:
                    bg.pop()()

        def stage_c(wp, sel):
            with (
                tc.tile_pool(name="stC", bufs=1) as C_sb,
                tc.tile_pool(name="stC2", bufs=2) as C_db,
                tc.tile_pool(name="psC", bufs=2, space="PSUM") as psC,
            ):
                nc.gpsimd.collective_compute(
                    "AllToAll", mybir.AluOpType.bypass,
                    replica_groups=[list(range(NCORES))],
                    ins=[a2a_in[:]], outs=[a2a_out[:]])
                a2a_b = a2a_out.bitcast(BF16)    # [src, h, 65, 256]
                ga = C_sb.tile([CH, NCORES, 256], BF16, tag="ga")
                dn = C_sb.tile([NHEADS, 256], BF16, tag="dn")
                for r in range(NCORES):
                    for h in range(HPC):
                        nc.sync.dma_start(
                            out=ga[h * HD:(h + 1) * HD, r, :],
                            in_=a2a_b[r, h, 0:HD, :])
                        nc.sync.dma_start(
                            out=dn[r * HPC + h:r * HPC + h + 1, :],
                            in_=a2a_b[r, h, HD:HD + 1, :])
                rec = C_sb.tile([NHEADS, 256], BF16, tag="rec")
                with nc.allow_low_precision(
                        reason="softmax denom reciprocal; bf16 suffices"):
                    nc.vector.reciprocal(rec[:], dn[:])
                # per-src: broadcast 1/denom to the 128 channel rows via a
                # selector matmul, then normalize ga in place.
                on2 = C_sb.tile([CH, NCORES, 256], BF16, tag="on2")
                for src in range(NCORES):
                    fac = psC.tile([128, 256], F32, tag="fac")
                    nc.tensor.matmul(fac[:], sel[:, src, :], rec[:],
                                     start=True, stop=True)
                    nc.vector.tensor_mul(on2[:, src, :], fac[:],
                                         ga[:, src, :])
                for nt in range(2):
                    for oc in range(2):
                        pp = psC.tile([128, 512], F32, tag="pp")
                        for src in range(NCORES):
                            nc.tensor.matmul(
                                pp[:], on2[:, src, nt * 128:(nt + 1) * 128],
                                wp[:, src, oc * 512:(oc + 1) * 512],
                                start=(src == 0), stop=(src == NCORES - 1))
                        ob = C_db.tile([128, 512], F32, tag="ob")
                        nc.vector.tensor_copy(ob[:], pp[:])
                        nc.sync.dma_start(
                            out=out_ext[nt * 128:(nt + 1) * 128,
                                        oc * 512:(oc + 1) * 512],
                            in_=ob[:])

        with tc.tile_pool(name="persist", bufs=1) as P1:
            Qp = P1.tile([CH, SEQ], BF16, tag="Qp")
            Kp = P1.tile([CH, SEQ], BF16, tag="Kp")
            Vsb = P1.tile([128, NKT, HPC, HD + 1], BF16, tag="Vsb")
            sel = P1.tile([NHEADS, NCORES, 128], BF16, tag="sel")
            xt = P1.tile([128, DCH, SEQ], BF16, tag="xt")
            wp = (P1.tile([128, DCH, DIM], BF16, tag="wp", name="wp")
                  if with_c else None)
            if nrep == 1:
                stage_ab(Qp, Kp, Vsb, sel, xt, wp, parts)
                if with_c:
                    stage_c(wp, sel)
            else:
                # timing build: loop stages A+B (a collective inside a For_i
                # desyncs the mesh), run stage C once after the loop.
                with tc.For_i(0, nrep, 1) as _i:
                    stage_ab(Qp, Kp, Vsb, sel, xt, wp, parts)
                if with_c:
                    stage_c(wp, sel)

    nc.compile()
    return nc


def _get_nc(nrep=1, n_cores=NCORES, with_c=True, parts="ab"):
    key = ("nc", nrep, n_cores, with_c, parts)
    if key not in _CACHE:
        _CACHE[key] = _build(nrep, n_cores, with_c, parts)
    return _CACHE[key]


def _prep_in_maps(x, wq, wk, wv, wp):
    import ml_dtypes

    bf16 = ml_dtypes.bfloat16
    x2 = np.ascontiguousarray(np.asarray(x, np.float32).reshape(SEQ, DIM))
    xt = np.ascontiguousarray(x2.T).astype(bf16)
    wq = np.asarray(wq, np.float64)
    wk = np.asarray(wk, np.float64)
    wv = np.asarray(wv, np.float64)
    wp = np.asarray(wp, np.float32)
    cos2, s2 = _rope_tables()
    scale = 1.0 / np.sqrt(HD)
    wq = wq * scale
    ck = np.ascontiguousarray(cos2).astype(bf16)
    sk = np.ascontiguousarray(s2).astype(bf16)
    r2t = _r2t().astype(bf16)
    sel = np.zeros((NCORES, NHEADS, 128), np.float32)
    for src in range(NCORES):
        for p in range(128):
            sel[src, src * HPC + p // HD, p] = 1.0
    sel = sel.astype(bf16)
    wpt = np.ascontiguousarray(wp.T)
    def wlay(w_t):
        # [DIM, CH] -> [128 part, DCH*CH] so the DMA is contiguous
        return np.ascontiguousarray(
            w_t.reshape(DCH, 128, CH).transpose(1, 0, 2).reshape(
                128, DCH * CH)).astype(bf16)

    wp_l = np.ascontiguousarray(
        wpt.reshape(DCH, 128, DIM).transpose(1, 0, 2).reshape(
            128, DCH * DIM)).astype(bf16)
    maps = []
    for c in range(NCORES):
        ch = slice(c * CH, (c + 1) * CH)
        maps.append({
            "xt": xt,
            "wq_t": wlay(np.ascontiguousarray(wq[ch, :].T)),
            "wk_t": wlay(np.ascontiguousarray(wk[ch, :].T)),
            "wv_t": wlay(np.ascontiguousarray(wv[ch, :].T)),
            "wp_t": wp_l,
            "cos_k": ck, "sin_k": sk,
            "r2t": r2t, "sel": sel,
        })
    return maps


def kernel(x, wq, wk, wv, wp):
    from concourse.bass_utils import run_bass_kernel_spmd

    nc = _get_nc(1)
    maps = _prep_in_maps(x, wq, wk, wv, wp)
    res = run_bass_kernel_spmd(nc, maps, list(range(NCORES))).results
    out = np.concatenate([res[c]["out"] for c in range(NCORES)], axis=0)
    return out.reshape(1, SEQ, DIM).astype(np.float32)
